# revision 87
# baseline (speedup 1.0000x reference)
"""Trainium2 Bass kernel for sliding-window causal self-attention.

Reference computation (B=1, T=4096, H=8 heads, head_dim=128, DIM=1024):
  qkv = x @ w_qkv.T; q,k = rms_norm -> rope; v = lam0*rms_norm(v) + lam1*ve
  scores = (q k^T) * 0.12 with sliding-window causal mask (0 <= i-j < 512)
  y = softmax(scores) @ v;  out = y @ o_w.T

Sharding over 8 cores: 2 sequence halves (S) x 4 head-pair groups (G).
Core c = 4*s + g handles t in [2048s, 2048(s+1)) for heads {2g, 2g+1}.
Each core reads its x rows plus a 512-row halo of preceding rows (for k/v),
computes its partial output projection over its 2 heads, and the host sums
the 4 partials per half and concatenates the halves. No on-chip collectives.

v2: single fused software-pipelined loop over 128-row t-chunks. Attention
pair pr (256 queries x 768-key window) fires at iteration 2pr+7; the output
projection window spreads two column-chunks per iteration. Chunk transposes
lag their chunk by two iterations so the PE never waits on the elementwise
chain, and the raw qkv psum is evacuated to SBUF by one Act copy so the
psum slot frees early (its release otherwise pins the pipeline period).
RMS rsqrt is the bit-trick seed + two stt-fused Newton steps on DVE —
sqrt/ln activations would force a 1283ns ACT_TABLE reload per switch
(Square/Exp/Copy share one table). Probabilities and v stay f32r (bf16
blows the 2e-2 error budget); ve arrives host-pre-scaled by lambda1 in
bf16; the softmax-denominator ones vector is a per-chunk column with zeros
in padded halo rows, absorbing the padding correction into the sums
matmul; output stores issue from Act (HWDGE) and Pool (SWDGE) so their
waits never block the SP input-prefetch queue.
"""

import sys

sys.path.insert(0, "/opt/trn_rl_repo")

import numpy as np
import ml_dtypes

import concourse.bass as bass
import concourse.mybir as mybir
import concourse.tile as tile
from concourse import bacc
from concourse.bass_utils import run_bass_kernel_spmd
from concourse.masks import make_identity

# Problem constants
T = 4096
DIM = 1024
H = 8
HD = 128
WINDOW = 512
ATTN_SCALE = 0.12
ROPE_BASE = 1024.0
EPS = 1e-6

# Sharding
S = 2          # sequence halves
G = 4          # head groups (2 heads each)
HPC = H // G   # heads per core = 2
TC = T // S    # own rows per core = 2048
TK = TC + WINDOW  # k/v rows incl. halo = 2560
NQT = TC // 128   # q tiles per head = 16
NKC = TK // 128   # k chunks = 20
NPR = TC // 256   # q pairs per head = 8
PW = 256 + WINDOW  # pair window = 768
NPC = PW // 128    # chunks per pair window = 6
EW = 3 * HPC * HD  # fused qkv width per core = 768

F32 = mybir.dt.float32
F32R = mybir.dt.float32r
BF16 = mybir.dt.bfloat16
I32 = mybir.dt.int32

AOP = mybir.AluOpType
AF = mybir.ActivationFunctionType
AX = mybir.AxisListType

BF = ml_dtypes.bfloat16
XW_BF16 = False
XW_DT = mybir.dt.bfloat16 if XW_BF16 else mybir.dt.float32r
XW_NP = ml_dtypes.bfloat16 if XW_BF16 else np.float32
PM_BF16 = False
PM_DT = mybir.dt.bfloat16 if PM_BF16 else mybir.dt.float32r
PM_MASK_DT = mybir.dt.bfloat16 if PM_BF16 else mybir.dt.float32


def build_kernel(debug=False):
    nc = bacc.Bacc()

    # Per-core DRAM I/O (shapes identical across cores; data differs).
    xT = nc.declare_dram_parameter("xT", [DIM, TK], XW_DT, isOutput=False)
    wqkvT = nc.declare_dram_parameter("wqkvT", [DIM, EW], XW_DT, isOutput=False)
    woT = nc.declare_dram_parameter("woT", [HPC * HD, DIM], F32, isOutput=False)
    ve = nc.declare_dram_parameter("ve", [TK, HPC * HD], BF16, isOutput=False)
    cosT = nc.declare_dram_parameter("cosT", [TK, 32], F32, isOutput=False)
    sinT = nc.declare_dram_parameter("sinT", [TK, 32], F32, isOutput=False)
    lam = nc.declare_dram_parameter("lam", [128, 1], F32, isOutput=False)
    onescol = nc.declare_dram_parameter("onescol", [128, NKC], PM_MASK_DT, isOutput=False)
    bandmask = nc.declare_dram_parameter("bandmask", [128, 4 * 256], PM_MASK_DT,
                                         isOutput=False)
    outT = nc.declare_dram_parameter("outT", [DIM, TC], F32, isOutput=True)
    dbg = None
    if debug:
        dbg = {
            "qT": [nc.declare_dram_parameter(f"dbg_qT{h}", [128, TC], F32,
                                             isOutput=True) for h in range(HPC)],
            "kT": [nc.declare_dram_parameter(f"dbg_kT{h}", [128, TK], F32,
                                             isOutput=True) for h in range(HPC)],
            "vbf": [nc.declare_dram_parameter(f"dbg_vbf{h}", [NKC * 128, HD],
                                              F32, isOutput=True)
                    for h in range(HPC)],
            "yT": [nc.declare_dram_parameter(f"dbg_yT{h}", [128, TC], F32,
                                             isOutput=True) for h in range(HPC)],
            "pm": nc.declare_dram_parameter("dbg_pm", [128, NPC * 256], F32,
                                            isOutput=True),
            "sums": nc.declare_dram_parameter("dbg_sums", [1, 256], F32,
                                              isOutput=True),
        }

    with tile.TileContext(nc) as tc:
        _trace_body(nc, tc, xT, wqkvT, woT, ve, cosT, sinT, lam, onescol,
                    bandmask, outT, dbg)

    nc.compile()
    return nc


def _trace_body(nc, tc, xT, wqkvT, woT, ve, cosT, sinT, lam, onescol,
                bandmask, outT, dbg=None):
    import contextlib

    ctx = contextlib.ExitStack()
    with ctx:
        const = ctx.enter_context(tc.tile_pool(name="const", bufs=1))
        persist = ctx.enter_context(tc.tile_pool(name="persist", bufs=1))

        # ---- weights, split per d-chunk so projection can start early ----
        w_sb = const.tile([128, 8, EW], XW_DT)  # wqkvT as [dpart, dchunk, e]
        wq_r = wqkvT.rearrange("(a p) e -> p a e", p=128)

        cos_sb = const.tile([128, NKC, 32], F32)
        sin_sb = const.tile([128, NKC, 32], F32)
        lam_sb = const.tile([128, 1], F32)
        ones_sb = const.tile([128, NKC], PM_DT)

        identity = const.tile([128, 128], F32R)
        idf = const.tile([128, 128], F32)
        make_identity(nc, idf)
        nc.vector.tensor_copy(out=identity, in_=idf)

        wo_sb = const.tile([128, HPC, DIM], F32R)  # woT as [ddpart, head, e]

        # Band masks in [kj, ci, qi] orientation for pair-window chunks,
        # host-provided. Chunk c of a pair window is valid iff
        # qi+1 <= 128c + kj <= qi+512. Chunks 2,3 are always fully valid;
        # 0,1 need the lower bound (maskA) and 4,5 the upper (maskB).
        maskAB = const.tile([128, 4, 256], PM_MASK_DT)
        maskA = maskAB[:, 0:2, :]
        maskB = maskAB[:, 2:4, :]

        # ---- persistent activations ----
        # qT/kT: [dd, t] per head (f32r); vbf: [t(kj) part, chunk, dd] bf16;
        # yT: [dd, t] f32r.
        qT = [persist.tile([128, TC], F32R, name=f"qT{h}") for h in range(HPC)]
        kT = [persist.tile([128, TK], F32R, name=f"kT{h}") for h in range(HPC)]
        vbf = [persist.tile([128, NKC, HD], F32R, name=f"vbf{h}") for h in range(HPC)]
        yT = [persist.tile([128, TC], F32R, name=f"yT{h}") for h in range(HPC)]

        xT_r = xT.rearrange("(a p) t -> p a t", p=128)  # [128, 8, TK]
        ve_r = ve.rearrange("(a p) d -> p a d", p=128)  # [128, 20, 256]

        with (
            tc.tile_pool(name="xt_pool", bufs=4) as xt_pool,
            tc.tile_pool(name="ve_pool", bufs=2) as ve_pool,
            tc.tile_pool(name="stage", bufs=3) as stage,
            tc.tile_pool(name="small", bufs=6) as small,
            tc.tile_pool(name="pm_pool", bufs=2) as pm_pool,
            tc.tile_pool(name="smallB", bufs=4) as smallB,
            tc.tile_pool(name="o_out", bufs=4) as o_out,
            tc.tile_pool(name="proj_psum", bufs=2, space="PSUM") as proj_psum,
            tc.tile_pool(name="sc_psum", bufs=2, space="PSUM") as sc_psum,
            tc.tile_pool(name="yo_psum", bufs=2, space="PSUM") as yo_psum,
        ):
            xt_tiles = {}
            st_tiles = {}

            def load_x(cc):
                # two t-chunks per load: bf16 rows below 512B pay a 2x DMA
                # descriptor penalty, so fetch 256 columns at a time
                xt = xt_pool.tile([128, 8, 256], XW_DT, name="xt", tag="xt")
                nc.sync.dma_start(out=xt, in_=xT_r[:, :, cc * 256:(cc + 1) * 256])
                xt_tiles[cc] = xt

            def load_ve(tb):
                vet = ve_pool.tile([128, 4, HPC * HD], BF16, name="vet", tag="ve")
                nc.sync.dma_start(out=vet, in_=ve_r[:, tb * 4:(tb + 1) * 4, :])
                return vet

            def proj_chunk(c):
                # fused qkv projection for t rows [128c, 128c+128)
                xt = xt_tiles[c // 2] if c % 2 == 0 else xt_tiles.pop(c // 2)
                lo = (c % 2) * 128
                psum = proj_psum.tile([128, EW], F32, name="psum", tag="proj")
                for dch in range(8):
                    lhsT = xt[:, dch, lo:lo + 128]
                    if c >= 4:
                        nc.tensor.matmul(
                            psum[:, 0:512], lhsT, w_sb[:, dch, 0:512],
                            start=(dch == 0), stop=(dch == 7),
                        )
                    else:  # halo rows need only k,v
                        nc.tensor.matmul(
                            psum[:, 256:512], lhsT, w_sb[:, dch, 256:512],
                            start=(dch == 0), stop=(dch == 7),
                        )
                    nc.tensor.matmul(
                        psum[:, 512:EW], lhsT, w_sb[:, dch, 512:EW],
                        start=(dch == 0), stop=(dch == 7),
                    )
                return psum

            def elem_chunk(c, psum, vet, tsub):
                # norm + rope for chunk c; writes st (q,k) and vbf (v)
                s0 = 0 if c >= 4 else 2
                psum6 = psum.rearrange("p (s d) -> p s d", s=6)

                # mean-square per segment: one batched Square + one reduce
                sq = stage.tile([128, 6, HD], BF16, name="sq", tag="sq")
                nc.scalar.activation(sq[:, s0:6, :], psum6[:, s0:6, :], AF.Square)
                # evacuate raw qkv to SBUF so the psum slot frees after two
                # fast Act ops instead of holding through the whole
                # reduce->rsqrt->norm chain (it pins the pipeline period)
                praw = stage.tile([128, 6, HD], F32, name="praw", tag="praw")
                nc.scalar.copy(out=praw[:, s0:6, :], in_=psum6[:, s0:6, :])
                psum6 = praw
                ssum = small.tile([128, 6], F32, name="ssum")
                nc.vector.tensor_reduce(
                    out=ssum[:, s0:6], in_=sq[:, s0:6, :], axis=AX.X,
                    op=AOP.add,
                )
                # rs = rsqrt(ssum) = rsqrt(msq)/sqrt(HD) via the classic
                # bit-trick seed + one Newton step (max rel err 0.18% at any
                # magnitude) — ALU-only, so Act stays on the {square, exp,
                # copy} table (sqrt/ln would force a 1283ns ACT_TABLE reload
                # per switch). The missing sqrt(HD) factor on q,k is folded
                # into the attention exp scale; for v into the host lam
                # value (lam0*sqrt(HD)). Pad rows clamp to eps -> finite.
                u_t = small.tile([128, 6], F32, name="u_t")
                uu = u_t[:, s0:6]
                nc.vector.tensor_scalar(out=uu, in0=ssum[:, s0:6],
                                        scalar1=HD * EPS, scalar2=None,
                                        op0=AOP.max)
                s_t = small.tile([128, 6], F32, name="s_t")
                ss_ = s_t[:, s0:6]
                si = s_t.bitcast(I32)[:, s0:6]
                nc.vector.tensor_scalar(out=si, in0=u_t.bitcast(I32)[:, s0:6],
                                        scalar1=1, scalar2=None,
                                        op0=AOP.logical_shift_right)
                nc.vector.tensor_scalar(out=si, in0=si, scalar1=-1,
                                        scalar2=0x5F3759DF, op0=AOP.mult,
                                        op1=AOP.add)
                # two Newton steps, stt-fused, all on DVE (cross-engine hops
                # on this chain stall the transposes two iterations later)
                t_t = small.tile([128, 6], F32, name="t_t")
                tt_ = t_t[:, s0:6]
                nc.vector.tensor_tensor(out=tt_, in0=ss_, in1=ss_, op=AOP.mult)
                nc.vector.scalar_tensor_tensor(out=tt_, in0=tt_, scalar=-0.5,
                                               in1=uu, op0=AOP.mult,
                                               op1=AOP.mult)
                nc.vector.scalar_tensor_tensor(out=ss_, in0=tt_, scalar=1.5,
                                               in1=ss_, op0=AOP.add,
                                               op1=AOP.mult)
                nc.vector.tensor_tensor(out=tt_, in0=ss_, in1=ss_, op=AOP.mult)
                nc.vector.scalar_tensor_tensor(out=tt_, in0=tt_, scalar=-0.5,
                                               in1=uu, op0=AOP.mult,
                                               op1=AOP.mult)
                rs = small.tile([128, 6], F32, name="rs")
                nc.vector.scalar_tensor_tensor(out=rs[:, s0:6], in0=tt_,
                                               scalar=1.5, in1=ss_,
                                               op0=AOP.add, op1=AOP.mult)
                nc.vector.tensor_scalar(out=rs[:, 4:6], in0=rs[:, 4:6],
                                        scalar1=lam_sb, scalar2=None,
                                        op0=AOP.mult)

                # normalize q,k into staging (f32r)
                st = stage.tile([128, 4, HD], F32R, name="st", tag="st")
                nc.vector.tensor_tensor(
                    out=st[:, s0:4, :], in0=psum6[:, s0:4, :],
                    in1=rs[:, s0:4, None].to_broadcast([128, 4 - s0, HD]),
                    op=AOP.mult,
                )
                stf = st.bitcast(F32)

                # v = lam0 * v/rms_v + lam1*ve (ve pre-scaled by host)
                for h in range(HPC):
                    nc.vector.scalar_tensor_tensor(
                        out=vbf[h][:, c, :], in0=psum6[:, 4 + h, :],
                        scalar=rs[:, 4 + h:5 + h], in1=vet[:, tsub, h * HD:(h + 1) * HD],
                        op0=AOP.mult, op1=AOP.add,
                    )

                # rope on q,k (dims 0:32 rotate with dims 64:96)
                nseg = 4 - s0
                cs = cos_sb[:, c:c + 1, :].to_broadcast([128, nseg, 32])
                sn = sin_sb[:, c:c + 1, :].to_broadcast([128, nseg, 32])
                x1 = stf[:, s0:4, 0:32]
                x2 = stf[:, s0:4, 64:96]
                t1 = stage.tile([128, 4, 32], F32, name="t1", tag="t1")
                t2 = stage.tile([128, 4, 32], F32, name="t2", tag="t2")
                t3 = stage.tile([128, 4, 32], F32, name="t3", tag="t3")
                t4 = stage.tile([128, 4, 32], F32, name="t4", tag="t4")
                nc.vector.tensor_tensor(out=t1[:, s0:4, :], in0=x1, in1=cs, op=AOP.mult)
                nc.vector.tensor_tensor(out=t2[:, s0:4, :], in0=x2, in1=sn, op=AOP.mult)
                nc.gpsimd.tensor_tensor(out=t3[:, s0:4, :], in0=x1, in1=sn, op=AOP.mult)
                nc.gpsimd.tensor_tensor(out=t4[:, s0:4, :], in0=x2, in1=cs, op=AOP.mult)
                nc.gpsimd.tensor_add(st[:, s0:4, 0:32], t1[:, s0:4, :], t2[:, s0:4, :])
                nc.gpsimd.tensor_sub(st[:, s0:4, 64:96], t4[:, s0:4, :], t3[:, s0:4, :])
                st_tiles[c] = st

            def transpose_chunk(c):
                # q,k of chunk c -> [dd, t] persistent buffers (f32r)
                st = st_tiles.pop(c)
                for h in range(HPC):
                    tk = sc_psum.tile([128, 128], F32R, name="tk", tag="sc")
                    nc.tensor.transpose(tk, st[:, 2 + h, :], identity)
                    nc.vector.tensor_copy(out=kT[h][:, c * 128:(c + 1) * 128],
                                          in_=tk)
                    if c >= 4:  # q exists only for own rows
                        tq = sc_psum.tile([128, 128], F32R, name="tq", tag="sc")
                        nc.tensor.transpose(tq, st[:, h, :], identity)
                        nc.scalar.copy(
                            out=qT[h][:, (c - 4) * 128:(c - 3) * 128], in_=tq)

            def attn_pair(pr):
                for h in range(HPC):
                    qs = qT[h][:, pr * 256:(pr + 1) * 256]
                    pm = pm_pool.tile([128, NPC, 256], PM_DT, name="pm", tag="pm")
                    # yv and sums accumulate interleaved groups; they must
                    # live in different PSUM banks (start=True zeroes the
                    # whole 2KB zero-region)
                    # sums allocated first: the next oproj part's psum then
                    # rotates into the sums slot (released at the recip,
                    # ~1us before yv's release at the yT evacuation)
                    sums_t = yo_psum.tile([128, 256], F32, name="sums",
                                          tag="yo")
                    sums = sums_t[0:1, :]
                    yv = yo_psum.tile([128, 256], F32, name="yv", tag="yo")
                    # masked chunk pairs first so the final accumulation
                    # tail has no mask op on its critical path
                    for i, wp in enumerate((0, 2, 1)):  # chunk pairs
                        sc = sc_psum.tile([128, 2, 256], F32, name="sc", tag="sc")
                        for j in range(2):
                            wc = 2 * wp + j
                            nc.tensor.matmul(
                                sc[:, j, :],
                                kT[h][:, (2 * pr + wc) * 128:(2 * pr + wc + 1) * 128],
                                qs, start=True, stop=True, skip_group_check=True,
                            )
                        nc.scalar.activation(pm[:, 2 * wp:2 * wp + 2, :], sc,
                                             AF.Exp, scale=ATTN_SCALE * HD)
                        if wp == 0:
                            nc.vector.tensor_tensor(
                                out=pm[:, 0:2, :], in0=pm[:, 0:2, :],
                                in1=maskA, op=AOP.mult)
                        elif wp == 2:
                            nc.vector.tensor_tensor(
                                out=pm[:, 4:6, :], in0=pm[:, 4:6, :],
                                in1=maskB, op=AOP.mult)
                        for j in range(2):
                            wc = 2 * wp + j
                            nc.tensor.matmul(
                                sums, ones_sb[:, 2 * pr + wc:2 * pr + wc + 1],
                                pm[:, wc, :],
                                start=(i == 0 and j == 0),
                                stop=(i == 2 and j == 1),
                                skip_group_check=True,
                            )
                            nc.tensor.matmul(
                                yv, vbf[h][:, 2 * pr + wc, :], pm[:, wc, :],
                                start=(i == 0 and j == 0),
                                stop=(i == 2 and j == 1),
                                skip_group_check=True,
                            )
                    if dbg is not None and pr == 0 and h == 0:
                        sumf = smallB.tile([1, 256], F32, name="sumf")
                        nc.vector.tensor_copy(out=sumf, in_=sums)
                        nc.sync.dma_start(out=dbg["sums"][:, :], in_=sumf)
                    with tc.high_priority(offset=40):
                        recip = smallB.tile([1, 256], F32, name="recip")
                        nc.vector.reciprocal(recip, sums)
                        # broadcast 1/sum across partitions on the Pool engine
                        bc_sb = smallB.tile([128, 256], F32, name="bc_sb")
                        nc.gpsimd.partition_broadcast(bc_sb, recip)
                    # evacuate with the 1/sum normalization fused (cast f32r)
                    nc.vector.tensor_tensor(
                        out=yT[h][:, pr * 256:(pr + 1) * 256],
                        in0=yv, in1=bc_sb, op=AOP.mult)

            def oproj_part(tw, part, half=None):
                # out[:, 512tw:512tw+512] = sum_h woT_h^T @ yT_h window;
                # two of the 8 output-row chunks per call so the store DMAs
                # never pile up waits on the SP queue ahead of x prefetches.
                # half=0/1 restricts to one 256-col pair block (tail split).
                t0, tn = tw * 512, 512
                if half is not None:
                    t0, tn = tw * 512 + half * 256, 256
                for ec in (2 * part, 2 * part + 1):
                    ops = yo_psum.tile([128, 512], F32, name="ops", tag="yo")
                    ops = ops[:, 0:tn]
                    for h in range(HPC):
                        nc.tensor.matmul(
                            ops,
                            wo_sb[:, h, ec * 128:(ec + 1) * 128],
                            yT[h][:, t0:t0 + tn],
                            start=(h == 0), stop=(h == HPC - 1),
                            skip_group_check=True,
                        )
                    # evac + store issued from the same engine: the store's
                    # wait is satisfied by the time it reaches the engine's
                    # queue, so it never blocks the queue behind it
                    ot = o_out.tile([128, 512], F32, name="ot")
                    ot = ot[:, 0:tn]
                    dst = outT[ec * 128:(ec + 1) * 128, t0:t0 + tn]
                    nc.scalar.copy(out=ot, in_=ops)
                    if ec % 2 == 0:
                        nc.scalar.dma_start(out=dst, in_=ot)
                    else:
                        # SWDGE: its wait parks in Pool's wait queue instead
                        # of blocking a sequencer
                        nc.gpsimd.dma_start(out=dst, in_=ot)

            # ---------------- fused pipeline ----------------
            # prologue DMAs, ordered by first use: x0 + weights gate chunk 0;
            # cos/sin gate chunk 0's rope; x2/x3 gate chunks 4..7 and must
            # not queue behind the big wo/mask loads (first used at c=7..9)
            load_x(0)
            # k/v weight columns first: halo chunks 0-3 need only cols
            # 256:768, so chunk 0 unblocks ~3us sooner; q columns land
            # well before chunk 4 (the first own chunk) needs them
            for dch in range(8):
                nc.sync.dma_start(out=w_sb[:, dch, 256:EW],
                                  in_=wq_r[:, dch, 256:EW])
            vet = load_ve(0)
            load_x(1)
            for dch in range(8):
                nc.sync.dma_start(out=w_sb[:, dch, 0:256],
                                  in_=wq_r[:, dch, 0:256])
            nc.sync.dma_start(
                out=cos_sb, in_=cosT.rearrange("(a p) f -> p a f", p=128))
            nc.sync.dma_start(
                out=sin_sb, in_=sinT.rearrange("(a p) f -> p a f", p=128))
            nc.sync.dma_start(out=lam_sb, in_=lam[:])
            load_x(2)
            load_x(3)
            nc.sync.dma_start(out=ones_sb, in_=onescol[:, :].bitcast(PM_DT))
            nc.sync.dma_start(
                out=maskAB,
                in_=bandmask.rearrange("p (a q) -> p a q", a=4))
            nc.sync.dma_start(
                out=wo_sb,
                in_=woT.rearrange("(a p) e -> p a e", p=128).bitcast(F32R))

            # warm the PE p-state during the DMA startup window
            for _ in range(14):
                wm = sc_psum.tile([128, 128], F32, name="wm", tag="sc")
                nc.tensor.matmul(wm, identity, identity, start=True, stop=True)

            # transposes lag their chunk by 2 iterations so the PE never
            # waits on the Square->reduce->rsqrt->norm->rope chain; pairs
            # and oproj windows shift accordingly
            for c in range(NKC):
                if c % 2 == 0 and c // 2 + 3 < NKC // 2:
                    load_x(c // 2 + 3)
                if c % 4 == 0 and c > 0:
                    vet = load_ve(c // 4)
                if c >= 2:
                    transpose_chunk(c - 2)
                psum = proj_chunk(c)
                elem_chunk(c, psum, vet, c % 4)
                if c >= 7 and c % 2 == 1:
                    attn_pair((c - 7) // 2)
                if c >= 9:
                    tw, part = divmod(c - 9, 4)
                    if tw <= 2 and not (tw == 2 and part == 3):
                        oproj_part(tw, part)

            # epilogue: last transposes + last pair; window 3 is emitted in
            # two 256-wide halves so the pair-6 half overlaps pair 7
            transpose_chunk(NKC - 2)
            transpose_chunk(NKC - 1)
            oproj_part(2, 3)
            attn_pair(NPR - 1)
            for part in range(4):
                oproj_part(3, part)

            if dbg is not None:
                with tc.tile_pool(name="dbgp", bufs=1) as dbgp:
                    for h in range(HPC):
                        for nm, t in (("qT", qT[h]), ("kT", kT[h]),
                                      ("yT", yT[h])):
                            n = t.shape[-1]
                            for blk in range((n + 1279) // 1280):
                                w = min(1280, n - blk * 1280)
                                f = dbgp.tile([128, 1280], F32, name=f"d{nm}",
                                              tag="dbgf")
                                nc.vector.tensor_copy(
                                    out=f[:, 0:w],
                                    in_=t[:, blk * 1280:blk * 1280 + w])
                                nc.sync.dma_start(
                                    out=dbg[nm][h][:, blk * 1280:blk * 1280 + w],
                                    in_=f[:, 0:w])
                        vr = dbg["vbf"][h].rearrange("(a p) d -> p a d", p=128)
                        for blk in range(2):
                            f = dbgp.tile([128, 10, HD], F32, name="dvb",
                                          tag="dbgf")
                            nc.vector.tensor_copy(
                                out=f, in_=vbf[h][:, blk * 10:(blk + 1) * 10, :])
                            nc.sync.dma_start(
                                out=vr[:, blk * 10:(blk + 1) * 10, :], in_=f)


_NC_CACHE = None


def _get_nc():
    global _NC_CACHE
    if _NC_CACHE is None:
        _NC_CACHE = build_kernel()
    return _NC_CACHE


def _rope_tables(positions):
    keep = HD // 4
    active = (1.0 / ROPE_BASE) ** np.linspace(0.0, 1.0, keep, dtype=np.float32)
    theta = positions[:, None].astype(np.float32) * active[None, :]  # [n, 32]
    return np.cos(theta).astype(np.float32), np.sin(theta).astype(np.float32)


def make_in_maps(x, ve, lambdas, qkvo_w):
    """Build the 8 per-core input maps from full inputs (host-side sharding)."""
    x2 = x.reshape(T, DIM)
    ve2 = ve.reshape(T, DIM)
    qw, kw, vw, ow = qkvo_w[0], qkvo_w[1], qkvo_w[2], qkvo_w[3]
    l0, l1 = float(lambdas[0]), float(lambdas[1])

    in_maps = []
    for c in range(8):
        s, g = divmod(c, G)
        h0, h1 = HPC * g, HPC * g + 1
        lo = TC * s - WINDOW  # first k/v row (may be negative -> zero pad)
        hi = TC * s + TC
        src_lo = max(lo, 0)

        # xT slice with zero pad (bf16)
        xs = np.zeros((TK, DIM), np.float32)
        xs[src_lo - lo:, :] = x2[src_lo:hi, :]
        xTc = np.ascontiguousarray(xs.T).astype(XW_NP)

        # fused qkv weight, transposed: cols = q0 q1 k0 k1 v0 v1 (bf16)
        wcols = []
        for wmat in (qw, kw, vw):
            for h in (h0, h1):
                wcols.append(wmat[h * HD:(h + 1) * HD, :].T)
        wqkvT = np.ascontiguousarray(np.concatenate(wcols, axis=1)).astype(XW_NP)

        woT = np.ascontiguousarray(ow[:, h0 * HD:(h1 + 1) * HD].T)

        # ve slice, pre-scaled by lambda1, bf16
        ves = np.zeros((TK, HPC * HD), np.float32)
        ves[src_lo - lo:, :] = ve2[src_lo:hi, h0 * HD:(h1 + 1) * HD] * l1
        ves = ves.astype(BF)

        pos = np.clip(np.arange(lo, hi), 0, None)
        cosT, sinT = _rope_tables(pos)

        lam = np.full((128, 1), l0 * np.sqrt(HD), np.float32)

        # softmax-denominator column: 0 for padded halo rows, 1 otherwise
        ones = np.ones((128, NKC), np.float32)
        if s == 0:
            npad = WINDOW  # rows [0, 512) of the k range are padding
            flat = np.ones(TK, np.float32)
            flat[:npad] = 0.0
            ones = flat.reshape(NKC, 128).T.copy()
        ones = ones.astype(BF if PM_BF16 else np.float32)

        # band masks [kj, chunk, qi]: chunk wc of a pair window is valid
        # iff qi+1 <= 128*wc + kj <= qi+512 (maskA: wc 0,1; maskB: wc 4,5)
        kj = np.arange(128)[:, None]
        qi = np.arange(256)[None, :]
        mask = np.zeros((128, 4, 256), np.float32)
        for i, wc in enumerate((0, 1, 4, 5)):
            mask[:, i, :] = ((qi + 1 <= 128 * wc + kj)
                             & (128 * wc + kj <= qi + 512))
        bandmask = mask.reshape(128, 4 * 256).astype(BF if PM_BF16 else np.float32)

        in_maps.append({
            "xT": xTc, "wqkvT": wqkvT, "woT": woT, "ve": ves,
            "cosT": cosT, "sinT": sinT, "lam": lam, "onescol": ones,
            "bandmask": bandmask,
        })
    return in_maps


def kernel(x, ve, lambdas, qkvo_w, window):
    assert int(window) == WINDOW
    x = np.asarray(x, np.float32)
    ve = np.asarray(ve, np.float32)
    lambdas = np.asarray(lambdas, np.float32)
    qkvo_w = np.asarray(qkvo_w, np.float32)

    nc = _get_nc()
    in_maps = make_in_maps(x, ve, lambdas, qkvo_w)
    res = run_bass_kernel_spmd(nc, in_maps, core_ids=list(range(8)))

    outT_full = np.zeros((DIM, T), np.float32)
    for c in range(8):
        s = c // G
        outT_full[:, TC * s:TC * (s + 1)] += res.results[c]["outT"]
    return np.ascontiguousarray(outT_full.T).reshape(1, T, DIM)


if __name__ == "__main__":
    nc = _get_nc()
    print("kernel built ok")


# revision 89
# speedup vs baseline: 1.0171x; 1.0171x over previous
"""Trainium2 Bass kernel for sliding-window causal self-attention.

Reference computation (B=1, T=4096, H=8 heads, head_dim=128, DIM=1024):
  qkv = x @ w_qkv.T; q,k = rms_norm -> rope; v = lam0*rms_norm(v) + lam1*ve
  scores = (q k^T) * 0.12 with sliding-window causal mask (0 <= i-j < 512)
  y = softmax(scores) @ v;  out = y @ o_w.T

Sharding over 8 cores: 2 sequence halves (S) x 4 head-pair groups (G).
Core c = 4*s + g handles t in [2048s, 2048(s+1)) for heads {2g, 2g+1}.
Each core reads its x rows plus a 512-row halo of preceding rows (for k/v),
computes its partial output projection over its 2 heads, and the host sums
the 4 partials per half and concatenates the halves. No on-chip collectives.

v2: single fused software-pipelined loop over 128-row t-chunks. Attention
pair pr (256 queries x 768-key window) fires at iteration 2pr+7; the output
projection window spreads two column-chunks per iteration. Chunk transposes
lag their chunk by two iterations so the PE never waits on the elementwise
chain, and the raw qkv psum is evacuated to SBUF by one Act copy so the
psum slot frees early (its release otherwise pins the pipeline period).
RMS rsqrt is the bit-trick seed + two stt-fused Newton steps on DVE —
sqrt/ln activations would force a 1283ns ACT_TABLE reload per switch
(Square/Exp/Copy share one table). Probabilities and v stay f32r (bf16
blows the 2e-2 error budget); ve arrives host-pre-scaled by lambda1 in
bf16; the softmax-denominator ones vector is a per-chunk column with zeros
in padded halo rows, absorbing the padding correction into the sums
matmul; output stores issue from Act (HWDGE) and Pool (SWDGE) so their
waits never block the SP input-prefetch queue.
"""

import sys

sys.path.insert(0, "/opt/trn_rl_repo")

import numpy as np
import ml_dtypes

import concourse.bass as bass
import concourse.mybir as mybir
import concourse.tile as tile
from concourse import bacc
from concourse.bass_utils import run_bass_kernel_spmd
from concourse.masks import make_identity

# Problem constants
T = 4096
DIM = 1024
H = 8
HD = 128
WINDOW = 512
ATTN_SCALE = 0.12
ROPE_BASE = 1024.0
EPS = 1e-6

# Sharding
S = 2          # sequence halves
G = 4          # head groups (2 heads each)
HPC = H // G   # heads per core = 2
TC = T // S    # own rows per core = 2048
TK = TC + WINDOW  # k/v rows incl. halo = 2560
NQT = TC // 128   # q tiles per head = 16
NKC = TK // 128   # k chunks = 20
NPR = TC // 256   # q pairs per head = 8
PW = 256 + WINDOW  # pair window = 768
NPC = PW // 128    # chunks per pair window = 6
EW = 3 * HPC * HD  # fused qkv width per core = 768

F32 = mybir.dt.float32
F32R = mybir.dt.float32r
BF16 = mybir.dt.bfloat16
I32 = mybir.dt.int32

AOP = mybir.AluOpType
AF = mybir.ActivationFunctionType
AX = mybir.AxisListType

BF = ml_dtypes.bfloat16
XW_BF16 = False
XW_DT = mybir.dt.bfloat16 if XW_BF16 else mybir.dt.float32r
XW_NP = ml_dtypes.bfloat16 if XW_BF16 else np.float32
PM_BF16 = False
PM_DT = mybir.dt.bfloat16 if PM_BF16 else mybir.dt.float32r
PM_MASK_DT = mybir.dt.bfloat16 if PM_BF16 else mybir.dt.float32


def build_kernel(debug=False):
    nc = bacc.Bacc()

    # Per-core DRAM I/O (shapes identical across cores; data differs).
    xT = nc.declare_dram_parameter("xT", [DIM, TK], XW_DT, isOutput=False)
    wqkvT = nc.declare_dram_parameter("wqkvT", [DIM, EW], XW_DT, isOutput=False)
    woT = nc.declare_dram_parameter("woT", [HPC * HD, DIM], F32, isOutput=False)
    ve = nc.declare_dram_parameter("ve", [TK, HPC * HD], BF16, isOutput=False)
    cosT = nc.declare_dram_parameter("cosT", [TK, 32], F32, isOutput=False)
    sinT = nc.declare_dram_parameter("sinT", [TK, 32], F32, isOutput=False)
    lam = nc.declare_dram_parameter("lam", [128, 1], F32, isOutput=False)
    onescol = nc.declare_dram_parameter("onescol", [128, NKC], PM_MASK_DT, isOutput=False)
    bandmask = nc.declare_dram_parameter("bandmask", [128, 4 * 256], PM_MASK_DT,
                                         isOutput=False)
    outT = nc.declare_dram_parameter("outT", [DIM, TC], F32, isOutput=True)
    dbg = None
    if debug:
        dbg = {
            "qT": [nc.declare_dram_parameter(f"dbg_qT{h}", [128, TC], F32,
                                             isOutput=True) for h in range(HPC)],
            "kT": [nc.declare_dram_parameter(f"dbg_kT{h}", [128, TK], F32,
                                             isOutput=True) for h in range(HPC)],
            "vbf": [nc.declare_dram_parameter(f"dbg_vbf{h}", [NKC * 128, HD],
                                              F32, isOutput=True)
                    for h in range(HPC)],
            "yT": [nc.declare_dram_parameter(f"dbg_yT{h}", [128, TC], F32,
                                             isOutput=True) for h in range(HPC)],
            "pm": nc.declare_dram_parameter("dbg_pm", [128, NPC * 256], F32,
                                            isOutput=True),
            "sums": nc.declare_dram_parameter("dbg_sums", [1, 256], F32,
                                              isOutput=True),
        }

    with tile.TileContext(nc) as tc:
        _trace_body(nc, tc, xT, wqkvT, woT, ve, cosT, sinT, lam, onescol,
                    bandmask, outT, dbg)

    nc.compile()
    return nc


def _trace_body(nc, tc, xT, wqkvT, woT, ve, cosT, sinT, lam, onescol,
                bandmask, outT, dbg=None):
    import contextlib

    ctx = contextlib.ExitStack()
    with ctx:
        const = ctx.enter_context(tc.tile_pool(name="const", bufs=1))
        persist = ctx.enter_context(tc.tile_pool(name="persist", bufs=1))

        # ---- weights, split per d-chunk so projection can start early ----
        w_sb = const.tile([128, 8, EW], XW_DT)  # wqkvT as [dpart, dchunk, e]
        wq_r = wqkvT.rearrange("(a p) e -> p a e", p=128)

        cos_sb = const.tile([128, NKC, 32], F32)
        sin_sb = const.tile([128, NKC, 32], F32)
        lam_sb = const.tile([128, 1], F32)
        ones_sb = const.tile([128, NKC], PM_DT)

        identity = const.tile([128, 128], F32R)
        idf = const.tile([128, 128], F32)
        make_identity(nc, idf)
        nc.vector.tensor_copy(out=identity, in_=idf)

        wo_sb = const.tile([128, HPC, DIM], F32R)  # woT as [ddpart, head, e]

        # Band masks in [kj, ci, qi] orientation for pair-window chunks,
        # host-provided. Chunk c of a pair window is valid iff
        # qi+1 <= 128c + kj <= qi+512. Chunks 2,3 are always fully valid;
        # 0,1 need the lower bound (maskA) and 4,5 the upper (maskB).
        maskAB = const.tile([128, 4, 256], PM_MASK_DT)
        maskA = maskAB[:, 0:2, :]
        maskB = maskAB[:, 2:4, :]

        # ---- persistent activations ----
        # qT/kT: [dd, t] per head (f32r); vbf: [t(kj) part, chunk, dd] bf16;
        # yT: [dd, t] f32r.
        qT = [persist.tile([128, TC], F32R, name=f"qT{h}") for h in range(HPC)]
        kT = [persist.tile([128, TK], F32R, name=f"kT{h}") for h in range(HPC)]
        vbf = [persist.tile([128, NKC, HD], F32R, name=f"vbf{h}") for h in range(HPC)]
        yT = [persist.tile([128, TC], F32R, name=f"yT{h}") for h in range(HPC)]

        xT_r = xT.rearrange("(a p) t -> p a t", p=128)  # [128, 8, TK]
        ve_r = ve.rearrange("(a p) d -> p a d", p=128)  # [128, 20, 256]

        with (
            tc.tile_pool(name="xt_pool", bufs=4) as xt_pool,
            tc.tile_pool(name="ve_pool", bufs=2) as ve_pool,
            tc.tile_pool(name="stage", bufs=3) as stage,
            tc.tile_pool(name="small", bufs=6) as small,
            tc.tile_pool(name="pm_pool", bufs=2) as pm_pool,
            tc.tile_pool(name="smallB", bufs=3) as smallB,
            tc.tile_pool(name="o_out", bufs=4) as o_out,
            tc.tile_pool(name="proj_psum", bufs=2, space="PSUM") as proj_psum,
            tc.tile_pool(name="sc_psum", bufs=2, space="PSUM") as sc_psum,
            tc.tile_pool(name="yo_psum", bufs=2, space="PSUM") as yo_psum,
        ):
            xt_tiles = {}
            st_tiles = {}

            def load_x(cc):
                # two t-chunks per load: bf16 rows below 512B pay a 2x DMA
                # descriptor penalty, so fetch 256 columns at a time
                xt = xt_pool.tile([128, 8, 256], XW_DT, name="xt", tag="xt")
                nc.sync.dma_start(out=xt, in_=xT_r[:, :, cc * 256:(cc + 1) * 256])
                xt_tiles[cc] = xt

            def load_ve(tb):
                vet = ve_pool.tile([128, 4, HPC * HD], BF16, name="vet", tag="ve")
                nc.sync.dma_start(out=vet, in_=ve_r[:, tb * 4:(tb + 1) * 4, :])
                return vet

            def proj_chunk(c):
                # fused qkv projection for t rows [128c, 128c+128)
                xt = xt_tiles[c // 2] if c % 2 == 0 else xt_tiles.pop(c // 2)
                lo = (c % 2) * 128
                psum = proj_psum.tile([128, EW], F32, name="psum", tag="proj")
                for dch in range(8):
                    lhsT = xt[:, dch, lo:lo + 128]
                    if c >= 4:
                        nc.tensor.matmul(
                            psum[:, 0:512], lhsT, w_sb[:, dch, 0:512],
                            start=(dch == 0), stop=(dch == 7),
                        )
                    else:  # halo rows need only k,v
                        nc.tensor.matmul(
                            psum[:, 256:512], lhsT, w_sb[:, dch, 256:512],
                            start=(dch == 0), stop=(dch == 7),
                        )
                    nc.tensor.matmul(
                        psum[:, 512:EW], lhsT, w_sb[:, dch, 512:EW],
                        start=(dch == 0), stop=(dch == 7),
                    )
                return psum

            def elem_chunk(c, psum, vet, tsub):
                # norm + rope for chunk c; writes st (q,k) and vbf (v)
                s0 = 0 if c >= 4 else 2
                psum6 = psum.rearrange("p (s d) -> p s d", s=6)

                # mean-square per segment: one batched Square + one reduce
                sq = stage.tile([128, 6, HD], BF16, name="sq", tag="sq")
                nc.scalar.activation(sq[:, s0:6, :], psum6[:, s0:6, :], AF.Square)
                # evacuate raw qkv to SBUF so the psum slot frees after two
                # fast Act ops instead of holding through the whole
                # reduce->rsqrt->norm chain (it pins the pipeline period)
                praw = stage.tile([128, 6, HD], F32, name="praw", tag="praw")
                nc.scalar.copy(out=praw[:, s0:6, :], in_=psum6[:, s0:6, :])
                psum6 = praw
                ssum = small.tile([128, 6], F32, name="ssum")
                nc.vector.tensor_reduce(
                    out=ssum[:, s0:6], in_=sq[:, s0:6, :], axis=AX.X,
                    op=AOP.add,
                )
                # rs = rsqrt(ssum) = rsqrt(msq)/sqrt(HD) via the classic
                # bit-trick seed + one Newton step (max rel err 0.18% at any
                # magnitude) — ALU-only, so Act stays on the {square, exp,
                # copy} table (sqrt/ln would force a 1283ns ACT_TABLE reload
                # per switch). The missing sqrt(HD) factor on q,k is folded
                # into the attention exp scale; for v into the host lam
                # value (lam0*sqrt(HD)). Pad rows clamp to eps -> finite.
                u_t = small.tile([128, 6], F32, name="u_t")
                uu = u_t[:, s0:6]
                nc.vector.tensor_scalar(out=uu, in0=ssum[:, s0:6],
                                        scalar1=HD * EPS, scalar2=None,
                                        op0=AOP.max)
                s_t = small.tile([128, 6], F32, name="s_t")
                ss_ = s_t[:, s0:6]
                si = s_t.bitcast(I32)[:, s0:6]
                nc.vector.tensor_scalar(out=si, in0=u_t.bitcast(I32)[:, s0:6],
                                        scalar1=1, scalar2=None,
                                        op0=AOP.logical_shift_right)
                nc.vector.tensor_scalar(out=si, in0=si, scalar1=-1,
                                        scalar2=0x5F3759DF, op0=AOP.mult,
                                        op1=AOP.add)
                # two Newton steps, stt-fused, all on DVE (cross-engine hops
                # on this chain stall the transposes two iterations later)
                t_t = small.tile([128, 6], F32, name="t_t")
                tt_ = t_t[:, s0:6]
                nc.vector.tensor_tensor(out=tt_, in0=ss_, in1=ss_, op=AOP.mult)
                nc.vector.scalar_tensor_tensor(out=tt_, in0=tt_, scalar=-0.5,
                                               in1=uu, op0=AOP.mult,
                                               op1=AOP.mult)
                nc.vector.scalar_tensor_tensor(out=ss_, in0=tt_, scalar=1.5,
                                               in1=ss_, op0=AOP.add,
                                               op1=AOP.mult)
                nc.vector.tensor_tensor(out=tt_, in0=ss_, in1=ss_, op=AOP.mult)
                nc.vector.scalar_tensor_tensor(out=tt_, in0=tt_, scalar=-0.5,
                                               in1=uu, op0=AOP.mult,
                                               op1=AOP.mult)
                rs = small.tile([128, 6], F32, name="rs")
                nc.vector.scalar_tensor_tensor(out=rs[:, s0:6], in0=tt_,
                                               scalar=1.5, in1=ss_,
                                               op0=AOP.add, op1=AOP.mult)
                nc.vector.tensor_scalar(out=rs[:, 4:6], in0=rs[:, 4:6],
                                        scalar1=lam_sb, scalar2=None,
                                        op0=AOP.mult)

                # normalize q,k into staging (f32r)
                st = stage.tile([128, 4, HD], F32R, name="st", tag="st")
                nc.vector.tensor_tensor(
                    out=st[:, s0:4, :], in0=psum6[:, s0:4, :],
                    in1=rs[:, s0:4, None].to_broadcast([128, 4 - s0, HD]),
                    op=AOP.mult,
                )
                stf = st.bitcast(F32)

                # v = lam0 * v/rms_v + lam1*ve (ve pre-scaled by host)
                for h in range(HPC):
                    nc.vector.scalar_tensor_tensor(
                        out=vbf[h][:, c, :], in0=psum6[:, 4 + h, :],
                        scalar=rs[:, 4 + h:5 + h], in1=vet[:, tsub, h * HD:(h + 1) * HD],
                        op0=AOP.mult, op1=AOP.add,
                    )

                # rope on q,k (dims 0:32 rotate with dims 64:96)
                nseg = 4 - s0
                cs = cos_sb[:, c:c + 1, :].to_broadcast([128, nseg, 32])
                sn = sin_sb[:, c:c + 1, :].to_broadcast([128, nseg, 32])
                x1 = stf[:, s0:4, 0:32]
                x2 = stf[:, s0:4, 64:96]
                t1 = stage.tile([128, 4, 32], F32, name="t1", tag="t1")
                t2 = stage.tile([128, 4, 32], F32, name="t2", tag="t2")
                t3 = stage.tile([128, 4, 32], F32, name="t3", tag="t3")
                t4 = stage.tile([128, 4, 32], F32, name="t4", tag="t4")
                nc.vector.tensor_tensor(out=t1[:, s0:4, :], in0=x1, in1=cs, op=AOP.mult)
                nc.vector.tensor_tensor(out=t2[:, s0:4, :], in0=x2, in1=sn, op=AOP.mult)
                nc.gpsimd.tensor_tensor(out=t3[:, s0:4, :], in0=x1, in1=sn, op=AOP.mult)
                nc.gpsimd.tensor_tensor(out=t4[:, s0:4, :], in0=x2, in1=cs, op=AOP.mult)
                nc.gpsimd.tensor_add(st[:, s0:4, 0:32], t1[:, s0:4, :], t2[:, s0:4, :])
                nc.gpsimd.tensor_sub(st[:, s0:4, 64:96], t4[:, s0:4, :], t3[:, s0:4, :])
                st_tiles[c] = st

            def transpose_chunk(c):
                # q,k of chunk c -> [dd, t] persistent buffers (f32r)
                st = st_tiles.pop(c)
                for h in range(HPC):
                    tk = sc_psum.tile([128, 128], F32R, name="tk", tag="sc")
                    nc.tensor.transpose(tk, st[:, 2 + h, :], identity)
                    nc.vector.tensor_copy(out=kT[h][:, c * 128:(c + 1) * 128],
                                          in_=tk)
                    if c >= 4:  # q exists only for own rows
                        tq = sc_psum.tile([128, 128], F32R, name="tq", tag="sc")
                        nc.tensor.transpose(tq, st[:, h, :], identity)
                        nc.scalar.copy(
                            out=qT[h][:, (c - 4) * 128:(c - 3) * 128], in_=tq)

            def attn_pair(pr):
                for h in range(HPC):
                    qs = qT[h][:, pr * 256:(pr + 1) * 256]
                    pm = pm_pool.tile([128, NPC, 256], PM_DT, name="pm", tag="pm")
                    # yv and sums accumulate interleaved groups; they must
                    # live in different PSUM banks (start=True zeroes the
                    # whole 2KB zero-region)
                    # sums allocated first: the next oproj part's psum then
                    # rotates into the sums slot (released at the recip,
                    # ~1us before yv's release at the yT evacuation)
                    sums_t = yo_psum.tile([128, 256], F32, name="sums",
                                          tag="yo")
                    sums = sums_t[0:1, :]
                    yv = yo_psum.tile([128, 256], F32, name="yv", tag="yo")
                    # masked chunk pairs first so the final accumulation
                    # tail has no mask op on its critical path
                    for i, wp in enumerate((0, 2, 1)):  # chunk pairs
                        sc = sc_psum.tile([128, 2, 256], F32, name="sc", tag="sc")
                        for j in range(2):
                            wc = 2 * wp + j
                            nc.tensor.matmul(
                                sc[:, j, :],
                                kT[h][:, (2 * pr + wc) * 128:(2 * pr + wc + 1) * 128],
                                qs, start=True, stop=True, skip_group_check=True,
                            )
                        nc.scalar.activation(pm[:, 2 * wp:2 * wp + 2, :], sc,
                                             AF.Exp, scale=ATTN_SCALE * HD)
                        if wp == 0:
                            nc.vector.tensor_tensor(
                                out=pm[:, 0:2, :], in0=pm[:, 0:2, :],
                                in1=maskA, op=AOP.mult)
                        elif wp == 2:
                            nc.vector.tensor_tensor(
                                out=pm[:, 4:6, :], in0=pm[:, 4:6, :],
                                in1=maskB, op=AOP.mult)
                        for j in range(2):
                            wc = 2 * wp + j
                            nc.tensor.matmul(
                                sums, ones_sb[:, 2 * pr + wc:2 * pr + wc + 1],
                                pm[:, wc, :],
                                start=(i == 0 and j == 0),
                                stop=(i == 2 and j == 1),
                                skip_group_check=True,
                            )
                            nc.tensor.matmul(
                                yv, vbf[h][:, 2 * pr + wc, :], pm[:, wc, :],
                                start=(i == 0 and j == 0),
                                stop=(i == 2 and j == 1),
                                skip_group_check=True,
                            )
                    if dbg is not None and pr == 0 and h == 0:
                        sumf = smallB.tile([1, 256], F32, name="sumf")
                        nc.vector.tensor_copy(out=sumf, in_=sums)
                        nc.sync.dma_start(out=dbg["sums"][:, :], in_=sumf)
                    # evacuate yv to SBUF immediately: its psum slot must not
                    # stay held through the recip/broadcast chain (the next
                    # oproj window's psum rotates into it)
                    yvf = smallB.tile([128, 256], F32, name="yvf")
                    nc.scalar.copy(out=yvf, in_=yv)
                    with tc.high_priority(offset=40):
                        recip = smallB.tile([1, 256], F32, name="recip")
                        nc.vector.reciprocal(recip, sums)
                        # broadcast 1/sum across partitions on the Pool engine
                        bc_sb = smallB.tile([128, 256], F32, name="bc_sb")
                        nc.gpsimd.partition_broadcast(bc_sb, recip)
                    # apply the 1/sum normalization (cast f32r)
                    nc.vector.tensor_tensor(
                        out=yT[h][:, pr * 256:(pr + 1) * 256],
                        in0=yvf, in1=bc_sb, op=AOP.mult)

            def oproj_part(tw, part, half=None):
                # out[:, 512tw:512tw+512] = sum_h woT_h^T @ yT_h window;
                # two of the 8 output-row chunks per call so the store DMAs
                # never pile up waits on the SP queue ahead of x prefetches.
                # half=0/1 restricts to one 256-col pair block (tail split).
                t0, tn = tw * 512, 512
                if half is not None:
                    t0, tn = tw * 512 + half * 256, 256
                for ec in (2 * part, 2 * part + 1):
                    ops = yo_psum.tile([128, 512], F32, name="ops", tag="yo")
                    ops = ops[:, 0:tn]
                    for h in range(HPC):
                        nc.tensor.matmul(
                            ops,
                            wo_sb[:, h, ec * 128:(ec + 1) * 128],
                            yT[h][:, t0:t0 + tn],
                            start=(h == 0), stop=(h == HPC - 1),
                            skip_group_check=True,
                        )
                    # evac + store issued from the same engine: the store's
                    # wait is satisfied by the time it reaches the engine's
                    # queue, so it never blocks the queue behind it
                    ot = o_out.tile([128, 512], F32, name="ot")
                    ot = ot[:, 0:tn]
                    dst = outT[ec * 128:(ec + 1) * 128, t0:t0 + tn]
                    nc.scalar.copy(out=ot, in_=ops)
                    if ec % 2 == 0:
                        nc.scalar.dma_start(out=dst, in_=ot)
                    else:
                        # SWDGE: its wait parks in Pool's wait queue instead
                        # of blocking a sequencer
                        nc.gpsimd.dma_start(out=dst, in_=ot)

            # ---------------- fused pipeline ----------------
            # prologue DMAs, ordered by first use: x0 + weights gate chunk 0;
            # cos/sin gate chunk 0's rope; x2/x3 gate chunks 4..7 and must
            # not queue behind the big wo/mask loads (first used at c=7..9)
            load_x(0)
            # k/v weight columns first: halo chunks 0-3 need only cols
            # 256:768, so chunk 0 unblocks ~3us sooner; q columns land
            # well before chunk 4 (the first own chunk) needs them
            for dch in range(8):
                nc.sync.dma_start(out=w_sb[:, dch, 256:EW],
                                  in_=wq_r[:, dch, 256:EW])
            vet = load_ve(0)
            load_x(1)
            for dch in range(8):
                nc.sync.dma_start(out=w_sb[:, dch, 0:256],
                                  in_=wq_r[:, dch, 0:256])
            nc.sync.dma_start(
                out=cos_sb, in_=cosT.rearrange("(a p) f -> p a f", p=128))
            nc.sync.dma_start(
                out=sin_sb, in_=sinT.rearrange("(a p) f -> p a f", p=128))
            nc.sync.dma_start(out=lam_sb, in_=lam[:])
            load_x(2)
            load_x(3)
            nc.sync.dma_start(out=ones_sb, in_=onescol[:, :].bitcast(PM_DT))
            nc.sync.dma_start(
                out=maskAB,
                in_=bandmask.rearrange("p (a q) -> p a q", a=4))
            nc.sync.dma_start(
                out=wo_sb,
                in_=woT.rearrange("(a p) e -> p a e", p=128).bitcast(F32R))

            # warm the PE p-state during the DMA startup window
            for _ in range(14):
                wm = sc_psum.tile([128, 128], F32, name="wm", tag="sc")
                nc.tensor.matmul(wm, identity, identity, start=True, stop=True)

            # transposes lag their chunk by 2 iterations so the PE never
            # waits on the Square->reduce->rsqrt->norm->rope chain; pairs
            # and oproj windows shift accordingly
            for c in range(NKC):
                if c % 2 == 0 and c // 2 + 3 < NKC // 2:
                    load_x(c // 2 + 3)
                if c % 4 == 0 and c > 0:
                    vet = load_ve(c // 4)
                if c >= 2:
                    transpose_chunk(c - 2)
                psum = proj_chunk(c)
                elem_chunk(c, psum, vet, c % 4)
                if c >= 7 and c % 2 == 1:
                    attn_pair((c - 7) // 2)
                if c >= 9:
                    tw, part = divmod(c - 9, 4)
                    if tw <= 2 and not (tw == 2 and part == 3):
                        oproj_part(tw, part)

            # epilogue: last transposes + last pair; window 3 is emitted in
            # two 256-wide halves so the pair-6 half overlaps pair 7
            transpose_chunk(NKC - 2)
            transpose_chunk(NKC - 1)
            oproj_part(2, 3)
            attn_pair(NPR - 1)
            for part in range(4):
                oproj_part(3, part)

            if dbg is not None:
                with tc.tile_pool(name="dbgp", bufs=1) as dbgp:
                    for h in range(HPC):
                        for nm, t in (("qT", qT[h]), ("kT", kT[h]),
                                      ("yT", yT[h])):
                            n = t.shape[-1]
                            for blk in range((n + 1279) // 1280):
                                w = min(1280, n - blk * 1280)
                                f = dbgp.tile([128, 1280], F32, name=f"d{nm}",
                                              tag="dbgf")
                                nc.vector.tensor_copy(
                                    out=f[:, 0:w],
                                    in_=t[:, blk * 1280:blk * 1280 + w])
                                nc.sync.dma_start(
                                    out=dbg[nm][h][:, blk * 1280:blk * 1280 + w],
                                    in_=f[:, 0:w])
                        vr = dbg["vbf"][h].rearrange("(a p) d -> p a d", p=128)
                        for blk in range(2):
                            f = dbgp.tile([128, 10, HD], F32, name="dvb",
                                          tag="dbgf")
                            nc.vector.tensor_copy(
                                out=f, in_=vbf[h][:, blk * 10:(blk + 1) * 10, :])
                            nc.sync.dma_start(
                                out=vr[:, blk * 10:(blk + 1) * 10, :], in_=f)


_NC_CACHE = None


def _get_nc():
    global _NC_CACHE
    if _NC_CACHE is None:
        _NC_CACHE = build_kernel()
    return _NC_CACHE


def _rope_tables(positions):
    keep = HD // 4
    active = (1.0 / ROPE_BASE) ** np.linspace(0.0, 1.0, keep, dtype=np.float32)
    theta = positions[:, None].astype(np.float32) * active[None, :]  # [n, 32]
    return np.cos(theta).astype(np.float32), np.sin(theta).astype(np.float32)


def make_in_maps(x, ve, lambdas, qkvo_w):
    """Build the 8 per-core input maps from full inputs (host-side sharding)."""
    x2 = x.reshape(T, DIM)
    ve2 = ve.reshape(T, DIM)
    qw, kw, vw, ow = qkvo_w[0], qkvo_w[1], qkvo_w[2], qkvo_w[3]
    l0, l1 = float(lambdas[0]), float(lambdas[1])

    in_maps = []
    for c in range(8):
        s, g = divmod(c, G)
        h0, h1 = HPC * g, HPC * g + 1
        lo = TC * s - WINDOW  # first k/v row (may be negative -> zero pad)
        hi = TC * s + TC
        src_lo = max(lo, 0)

        # xT slice with zero pad (bf16)
        xs = np.zeros((TK, DIM), np.float32)
        xs[src_lo - lo:, :] = x2[src_lo:hi, :]
        xTc = np.ascontiguousarray(xs.T).astype(XW_NP)

        # fused qkv weight, transposed: cols = q0 q1 k0 k1 v0 v1 (bf16)
        wcols = []
        for wmat in (qw, kw, vw):
            for h in (h0, h1):
                wcols.append(wmat[h * HD:(h + 1) * HD, :].T)
        wqkvT = np.ascontiguousarray(np.concatenate(wcols, axis=1)).astype(XW_NP)

        woT = np.ascontiguousarray(ow[:, h0 * HD:(h1 + 1) * HD].T)

        # ve slice, pre-scaled by lambda1, bf16
        ves = np.zeros((TK, HPC * HD), np.float32)
        ves[src_lo - lo:, :] = ve2[src_lo:hi, h0 * HD:(h1 + 1) * HD] * l1
        ves = ves.astype(BF)

        pos = np.clip(np.arange(lo, hi), 0, None)
        cosT, sinT = _rope_tables(pos)

        lam = np.full((128, 1), l0 * np.sqrt(HD), np.float32)

        # softmax-denominator column: 0 for padded halo rows, 1 otherwise
        ones = np.ones((128, NKC), np.float32)
        if s == 0:
            npad = WINDOW  # rows [0, 512) of the k range are padding
            flat = np.ones(TK, np.float32)
            flat[:npad] = 0.0
            ones = flat.reshape(NKC, 128).T.copy()
        ones = ones.astype(BF if PM_BF16 else np.float32)

        # band masks [kj, chunk, qi]: chunk wc of a pair window is valid
        # iff qi+1 <= 128*wc + kj <= qi+512 (maskA: wc 0,1; maskB: wc 4,5)
        kj = np.arange(128)[:, None]
        qi = np.arange(256)[None, :]
        mask = np.zeros((128, 4, 256), np.float32)
        for i, wc in enumerate((0, 1, 4, 5)):
            mask[:, i, :] = ((qi + 1 <= 128 * wc + kj)
                             & (128 * wc + kj <= qi + 512))
        bandmask = mask.reshape(128, 4 * 256).astype(BF if PM_BF16 else np.float32)

        in_maps.append({
            "xT": xTc, "wqkvT": wqkvT, "woT": woT, "ve": ves,
            "cosT": cosT, "sinT": sinT, "lam": lam, "onescol": ones,
            "bandmask": bandmask,
        })
    return in_maps


def kernel(x, ve, lambdas, qkvo_w, window):
    assert int(window) == WINDOW
    x = np.asarray(x, np.float32)
    ve = np.asarray(ve, np.float32)
    lambdas = np.asarray(lambdas, np.float32)
    qkvo_w = np.asarray(qkvo_w, np.float32)

    nc = _get_nc()
    in_maps = make_in_maps(x, ve, lambdas, qkvo_w)
    res = run_bass_kernel_spmd(nc, in_maps, core_ids=list(range(8)))

    outT_full = np.zeros((DIM, T), np.float32)
    for c in range(8):
        s = c // G
        outT_full[:, TC * s:TC * (s + 1)] += res.results[c]["outT"]
    return np.ascontiguousarray(outT_full.T).reshape(1, T, DIM)


if __name__ == "__main__":
    nc = _get_nc()
    print("kernel built ok")
